# revision 4
# baseline (speedup 1.0000x reference)
"""Trainium2 Bass kernel for nn_AttentionBlock (GroupNorm + single-head
attention over N=HW + 1x1 convs + residual).

Sharding: data-parallel over batch. B=16 across 8 cores -> 2 batch elements
per core, no collectives.

Per-core pipeline (per batch element, layouts chosen so no PE transposes are
ever needed):
  x        [C=512(part,4x128), N=1024(free)]  fp32
  GroupNorm: bn_stats/bn_aggr per channel, cross-partition group reduce via a
             tiny fp32 matmul with a 0/1 group-selection matrix, broadcast
             back via its transpose; h = x*a + b  (a,b per-channel) -> f32r
  qkv:      q,k as [C, N] = W^T-tiles.T @ h;  v directly transposed as
            vT [N, C] = h-tiles.T @ WvT  (free!)
  scores:   S^T [m, n] = k.T @ q  (lhsT=k tile, rhs=q)  -> PSUM
  softmax:  exp on ACT straight out of PSUM (scale folded into activation),
            no max-subtraction (scores are O(5), fp32 exp cannot overflow),
            row sums via ones-vector matmul (partition reduce on PE),
            reciprocal on DVE, broadcast via K=1 ones matmul
  PV:       out [C, N] = vT-chunks.T @ P^T  (both already in layout)
  proj:     y = WpT-tiles.T @ (out * recip) + pb_eff + x
All big matmuls in float32r (fp32 storage, TF32-like 11-bit-mantissa
multiply, full fp32 PSUM accumulation, 1 cycle/row).

kernel(**inputs) takes the FULL unsharded inputs and returns the full output.
"""
import numpy as np

import concourse.bacc as bacc
import concourse.tile as tile
from concourse import mybir
from concourse.bass_utils import run_bass_kernel_spmd

f32 = mybir.dt.float32
f32r = mybir.dt.float32r
AF = mybir.ActivationFunctionType
ALU = mybir.AluOpType

B, C, H, W = 16, 512, 32, 32
N = H * W                  # 1024
NCORES = 8
BPC = B // NCORES          # 2 batch elements per core
NG = 32                    # groups
GS = C // NG               # 16 channels per group
EPS = 1e-6
NCT = C // 128             # 4 channel tiles
NNT = N // 128             # 8 position tiles
NCHK = N // 512            # 2 free-dim chunks of 512
SCALE = float(C) ** -0.5


def _emit(nc, n_bodies):
    """Emit the kernel body. n_bodies batch-bodies are emitted cycling over
    the BPC batch slots (n_bodies == BPC for the real kernel; larger values
    are used only to build timing-amplified variants)."""
    x_d = nc.declare_dram_parameter("x", [BPC, C, N], f32, isOutput=False)
    wqkv_d = nc.declare_dram_parameter("wqkv", [C, 3 * C], f32r, isOutput=False)
    wp_d = nc.declare_dram_parameter("wp", [C, C], f32r, isOutput=False)
    qkb_d = nc.declare_dram_parameter("qkb", [128, 8], f32, isOutput=False)
    gns_d = nc.declare_dram_parameter("gns", [128, NCT], f32, isOutput=False)
    gnb_d = nc.declare_dram_parameter("gnb", [128, NCT], f32, isOutput=False)
    pbe_d = nc.declare_dram_parameter("pbe", [128, NCT], f32, isOutput=False)
    g_d = nc.declare_dram_parameter("gsel", [128, 8], f32, isOutput=False)
    gt_d = nc.declare_dram_parameter("gselT", [8, 128], f32, isOutput=False)
    y_d = nc.declare_dram_parameter("y", [BPC, C, N], f32, isOutput=True)

    from contextlib import ExitStack
    with tile.TileContext(nc) as tc, ExitStack() as ctx:
        sing = ctx.enter_context(tc.tile_pool(name="sing", bufs=1))
        big = ctx.enter_context(tc.tile_pool(name="big", bufs=1))
        gnp = ctx.enter_context(tc.tile_pool(name="gnp", bufs=2))
        mp = ctx.enter_context(tc.tile_pool(name="mp", bufs=4, space="PSUM"))
        stp = ctx.enter_context(tc.tile_pool(name="stp", bufs=2, space="PSUM"))
        sump = ctx.enter_context(tc.tile_pool(name="sump", bufs=1, space="PSUM"))
        rbp = ctx.enter_context(tc.tile_pool(name="rbp", bufs=1, space="PSUM"))

        # ---- persistent weights / constants ----
        wt = [sing.tile([128, 3 * C], f32r, tag=f"w{kt}", name=f"w{kt}")
              for kt in range(NCT)]
        wpt = [sing.tile([128, C], f32r, tag=f"wp{kt}", name=f"wp{kt}")
               for kt in range(NCT)]
        for kt in range(NCT):
            nc.sync.dma_start(out=wt[kt], in_=wqkv_d[kt * 128:(kt + 1) * 128, :])
            nc.sync.dma_start(out=wpt[kt], in_=wp_d[kt * 128:(kt + 1) * 128, :])
        qkb = sing.tile([128, 8], f32, tag="qkb", name="qkb")
        gns = sing.tile([128, NCT], f32, tag="gns", name="gns")
        gnb = sing.tile([128, NCT], f32, tag="gnb", name="gnb")
        pbe = sing.tile([128, NCT], f32, tag="pbe", name="pbe")
        g_t = sing.tile([128, 8], f32, tag="g_t", name="g_t")
        gt_t = sing.tile([8, 128], f32, tag="gt_t", name="gt_t")
        nc.sync.dma_start(out=qkb, in_=qkb_d[:, :])
        nc.sync.dma_start(out=gns, in_=gns_d[:, :])
        nc.sync.dma_start(out=gnb, in_=gnb_d[:, :])
        nc.sync.dma_start(out=pbe, in_=pbe_d[:, :])
        nc.sync.dma_start(out=g_t, in_=g_d[:, :])
        nc.sync.dma_start(out=gt_t, in_=gt_d[:, :])
        eps_t = sing.tile([128, 1], f32, tag="eps", name="eps")
        nc.vector.memset(eps_t, EPS)
        ones_f32 = sing.tile([128, 1], f32, tag="ones_f", name="ones_f")
        nc.vector.memset(ones_f32, 1.0)
        ones_row_f32 = sing.tile([1, 128], f32, tag="ones_rf", name="ones_rf")
        nc.vector.memset(ones_row_f32, 1.0)
        ones_col = sing.tile([128, 1], f32r, tag="ones_c", name="ones_c")
        nc.vector.tensor_copy(out=ones_col, in_=ones_f32)
        ones_row = sing.tile([1, 128], f32r, tag="ones_r", name="ones_r")
        nc.vector.tensor_copy(out=ones_row, in_=ones_row_f32)

        for body in range(n_bodies):
            b = body % BPC
            # ---- load x ----
            x_t = [big.tile([128, N], f32, tag=f"x{ct}", name=f"x{body}_{ct}")
                   for ct in range(NCT)]
            for ct in range(NCT):
                nc.sync.dma_start(out=x_t[ct],
                                  in_=x_d[b, ct * 128:(ct + 1) * 128, :])

            # ---- GroupNorm ----
            h_t = []
            for ct in range(NCT):
                st = gnp.tile([128, 2, 6], f32, tag="st", name=f"st{body}_{ct}")
                for sg in range(2):
                    nc.vector.bn_stats(out=st[:, sg, :],
                                       in_=x_t[ct][:, sg * 512:(sg + 1) * 512])
                mv = gnp.tile([128, 2], f32, tag="mv", name=f"mv{body}_{ct}")
                nc.vector.bn_aggr(out=mv, in_=st)
                # m1 = (mean, E[x^2]) per channel
                m1 = gnp.tile([128, 2], f32, tag="m1", name=f"m1{body}_{ct}")
                nc.vector.tensor_copy(out=m1[:, 0:1], in_=mv[:, 0:1])
                sqm = gnp.tile([128, 1], f32, tag="sqm", name=f"sqm{body}_{ct}")
                nc.vector.tensor_mul(out=sqm, in0=mv[:, 0:1], in1=mv[:, 0:1])
                nc.vector.tensor_add(out=m1[:, 1:2], in0=mv[:, 1:2], in1=sqm)
                # group sums over 16-channel blocks (cross-partition)
                gs_ps = stp.tile([8, 2], f32, tag="stat", name=f"gs{body}_{ct}")
                nc.tensor.matmul(gs_ps, g_t, m1, start=True, stop=True)
                gsb = gnp.tile([8, 2], f32, tag="gsb", name=f"gsb{body}_{ct}")
                nc.scalar.mul(out=gsb, in_=gs_ps, mul=1.0 / GS)
                # var_g = E[x^2] - mean^2 ; rstd = 1/sqrt(var+eps)
                t8 = gnp.tile([8, 1], f32, tag="t8", name=f"t8{body}_{ct}")
                nc.vector.tensor_mul(out=t8, in0=gsb[:, 0:1], in1=gsb[:, 0:1])
                vg = gnp.tile([8, 1], f32, tag="vg", name=f"vg{body}_{ct}")
                nc.vector.tensor_sub(out=vg, in0=gsb[:, 1:2], in1=t8)
                nc.scalar.activation(out=vg, in_=vg, func=AF.Sqrt,
                                     bias=eps_t[:8, :], scale=1.0)
                st2 = gnp.tile([8, 2], f32, tag="st2", name=f"st2{body}_{ct}")
                nc.vector.tensor_copy(out=st2[:, 0:1], in_=gsb[:, 0:1])
                nc.vector.reciprocal(out=st2[:, 1:2], in_=vg)
                # broadcast (mean_g, rstd_g) back to the 16 channels each
                bc_ps = stp.tile([128, 2], f32, tag="stat", name=f"bc{body}_{ct}")
                nc.tensor.matmul(bc_ps, gt_t, st2, start=True, stop=True)
                # a = rstd*gn_scale ; b = gn_bias - mean*a ; h = x*a + b
                a_sb = gnp.tile([128, 1], f32, tag="a_sb", name=f"a{body}_{ct}")
                nc.vector.tensor_mul(out=a_sb, in0=bc_ps[:, 1:2],
                                     in1=gns[:, ct:ct + 1])
                t1 = gnp.tile([128, 1], f32, tag="t1", name=f"t1{body}_{ct}")
                nc.vector.tensor_mul(out=t1, in0=bc_ps[:, 0:1], in1=a_sb)
                b_sb = gnp.tile([128, 1], f32, tag="b_sb", name=f"bb{body}_{ct}")
                nc.vector.tensor_sub(out=b_sb, in0=gnb[:, ct:ct + 1], in1=t1)
                ht = big.tile([128, N], f32r, tag=f"h{ct}", name=f"h{body}_{ct}")
                nc.vector.tensor_scalar(out=ht, in0=x_t[ct], scalar1=a_sb,
                                        scalar2=b_sb, op0=ALU.mult, op1=ALU.add)
                h_t.append(ht)

            # ---- qkv ----
            q_t = [big.tile([128, N], f32r, tag=f"q{ct}", name=f"q{body}_{ct}")
                   for ct in range(NCT)]
            k_t = [big.tile([128, N], f32r, tag=f"k{ct}", name=f"k{body}_{ct}")
                   for ct in range(NCT)]
            for o in range(2 * NCT):          # q: o 0..3, k: o 4..7
                dst = q_t[o] if o < NCT else k_t[o - NCT]
                for nch in range(NCHK):
                    sl = slice(nch * 512, (nch + 1) * 512)
                    ps = mp.tile([128, 512], f32, tag="mm", name=f"qk{body}_{o}_{nch}")
                    for kt in range(NCT):
                        nc.tensor.matmul(ps, wt[kt][:, o * 128:(o + 1) * 128],
                                         h_t[kt][:, sl],
                                         start=(kt == 0), stop=(kt == NCT - 1))
                    nc.vector.tensor_scalar_add(out=dst[:, sl], in0=ps,
                                                scalar1=qkb[:, o:o + 1])
            vT_t = [big.tile([128, C], f32r, tag=f"vT{nt}", name=f"vT{body}_{nt}")
                    for nt in range(NNT)]
            for nt in range(NNT):
                ps = mp.tile([128, 512], f32, tag="mm", name=f"v{body}_{nt}")
                for kt in range(NCT):
                    nc.tensor.matmul(ps, h_t[kt][:, nt * 128:(nt + 1) * 128],
                                     wt[kt][:, 2 * C:3 * C],
                                     start=(kt == 0), stop=(kt == NCT - 1))
                nc.vector.tensor_copy(out=vT_t[nt], in_=ps)

            # ---- scores + exp (S^T layout [m, n]) ----
            pT_t = [big.tile([128, N], f32r, tag=f"pT{mt}", name=f"pT{body}_{mt}")
                    for mt in range(NNT)]
            for mt in range(NNT):
                for nch in range(NCHK):
                    sl = slice(nch * 512, (nch + 1) * 512)
                    ps = mp.tile([128, 512], f32, tag="mm", name=f"s{body}_{mt}_{nch}")
                    for kt in range(NCT):
                        nc.tensor.matmul(ps, k_t[kt][:, mt * 128:(mt + 1) * 128],
                                         q_t[kt][:, sl],
                                         start=(kt == 0), stop=(kt == NCT - 1))
                    nc.scalar.activation(out=pT_t[mt][:, sl], in_=ps,
                                         func=AF.Exp, scale=SCALE)

            # ---- softmax denominators ----
            rb_sb = []
            for nch in range(NCHK):
                sl = slice(nch * 512, (nch + 1) * 512)
                sum_ps = sump.tile([1, 512], f32, tag="sums", name=f"sm{body}_{nch}")
                for mt in range(NNT):
                    nc.tensor.matmul(sum_ps, ones_col, pT_t[mt][:, sl],
                                     start=(mt == 0), stop=(mt == NNT - 1))
                rc = gnp.tile([1, 512], f32r, tag="rc", name=f"rc{body}_{nch}")
                with nc.allow_low_precision(reason="f32r feed for bcast matmul"):
                    nc.vector.reciprocal(out=rc, in_=sum_ps)
                rb_ps = rbp.tile([128, 512], f32, tag="rb", name=f"rbp{body}_{nch}")
                nc.tensor.matmul(rb_ps, ones_row, rc, start=True, stop=True)
                rb = gnp.tile([128, 512], f32, tag="rb_sb", name=f"rb{body}_{nch}")
                nc.vector.tensor_copy(out=rb, in_=rb_ps)
                rb_sb.append(rb)

            # ---- PV ----
            out_t = [big.tile([128, N], f32r, tag=f"o{ct}", name=f"o{body}_{ct}")
                     for ct in range(NCT)]
            for ct in range(NCT):
                for nch in range(NCHK):
                    sl = slice(nch * 512, (nch + 1) * 512)
                    ps = mp.tile([128, 512], f32, tag="mm", name=f"pv{body}_{ct}_{nch}")
                    for mt in range(NNT):
                        nc.tensor.matmul(ps, vT_t[mt][:, ct * 128:(ct + 1) * 128],
                                         pT_t[mt][:, sl],
                                         start=(mt == 0), stop=(mt == NNT - 1))
                    nc.vector.tensor_mul(out=out_t[ct][:, sl], in0=ps,
                                         in1=rb_sb[nch])

            # ---- proj + bias + residual ----
            for ot in range(NCT):
                fin = big.tile([128, N], f32, tag=f"fin{ot}", name=f"fin{body}_{ot}")
                for nch in range(NCHK):
                    sl = slice(nch * 512, (nch + 1) * 512)
                    ps = mp.tile([128, 512], f32, tag="mm", name=f"pj{body}_{ot}_{nch}")
                    for ct in range(NCT):
                        nc.tensor.matmul(ps, wpt[ct][:, ot * 128:(ot + 1) * 128],
                                         out_t[ct][:, sl],
                                         start=(ct == 0), stop=(ct == NCT - 1))
                    nc.scalar.activation(out=fin[:, sl], in_=ps, func=AF.Identity,
                                         bias=pbe[:, ot:ot + 1], scale=1.0)
                    nc.vector.tensor_add(out=fin[:, sl], in0=fin[:, sl],
                                         in1=x_t[ot][:, sl])
                nc.sync.dma_start(out=y_d[b, ot * 128:(ot + 1) * 128, :], in_=fin)


def build(n_bodies=BPC):
    nc = bacc.Bacc("TRN2")
    _emit(nc, n_bodies)
    nc.compile()
    return nc


_cached = {}


def get_nc(n_bodies=BPC):
    if n_bodies not in _cached:
        _cached[n_bodies] = build(n_bodies)
    return _cached[n_bodies]


def make_in_maps(x, gn_scale, gn_bias, qkv_w, qkv_b, proj_w, proj_b):
    x = np.ascontiguousarray(np.asarray(x, np.float32).reshape(B, C, N))
    gn_scale = np.asarray(gn_scale, np.float32)
    gn_bias = np.asarray(gn_bias, np.float32)
    qkv_w = np.asarray(qkv_w, np.float32)
    qkv_b = np.asarray(qkv_b, np.float32)
    proj_w = np.asarray(proj_w, np.float32)
    proj_b = np.asarray(proj_b, np.float32)

    wqkvT = np.ascontiguousarray(qkv_w.T)                      # [C, 3C]
    wpT = np.ascontiguousarray(proj_w.T)                       # [C, C]
    qkb = np.ascontiguousarray(qkv_b[:2 * C].reshape(8, 128).T)  # [128, 8]
    gns = np.ascontiguousarray(gn_scale.reshape(NCT, 128).T)
    gnb = np.ascontiguousarray(gn_bias.reshape(NCT, 128).T)
    pbe_vec = proj_w @ qkv_b[2 * C:] + proj_b                  # fold v-bias
    pbe = np.ascontiguousarray(pbe_vec.astype(np.float32).reshape(NCT, 128).T)
    gsel = np.zeros((128, 8), np.float32)
    gsel[np.arange(128), np.arange(128) // GS] = 1.0
    gselT = np.ascontiguousarray(gsel.T)

    shared = {"wqkv": wqkvT, "wp": wpT, "qkb": qkb, "gns": gns,
              "gnb": gnb, "pbe": pbe, "gsel": gsel, "gselT": gselT}
    return [{"x": np.ascontiguousarray(x[BPC * i:BPC * (i + 1)]), **shared}
            for i in range(NCORES)]


def kernel(x, gn_scale, gn_bias, qkv_w, qkv_b, proj_w, proj_b):
    in_maps = make_in_maps(x, gn_scale, gn_bias, qkv_w, qkv_b, proj_w, proj_b)
    nc = get_nc()
    res = run_bass_kernel_spmd(nc, in_maps, list(range(NCORES)))
    y = np.concatenate([res.results[i]["y"] for i in range(NCORES)], axis=0)
    return np.ascontiguousarray(y.reshape(B, C, H, W).astype(np.float32))
